# revision 9
# baseline (speedup 1.0000x reference)
"""CopyGenerator kernel for 8 Trainium2 NeuronCores.

Strategy: tensor-parallel over the vocab dimension.
  - Each core computes logits = hidden @ W[:, k*4000:(k+1)*4000] (bf16 matmul,
    fp32 accumulate), exp via ACT with fused row-sum (accum_out).
  - Softmax denominator: partial row-sums AllReduced across the 8 cores in
    pipelined chunks [6,6,3,1]; a warmup AllReduce absorbs cold-start cost.
  - Queue discipline: inputs split across three DGE queues (sync: W tiles,
    scalar: hT, vector: small tensors) for a fast ramp; gpsimd carries ONLY
    the row-sum adds / AR feeders / triggers so the last AR fires the moment
    its rows finish; z readbacks ride the sync queue right before the writes
    they gate.
  - PAD masking without bias matmuls: column 1's exp is zeroed via a
    per-core host mask and subtracted from the row-sum.
  - out_prob shard = e * (1 - p_copy) / Z written in bf16 (host upcasts).
  - p_copy prologue + copy-path head run right after chunk 0 (PE stays hot);
    the copy-path einsum runs inside the last AR's latency window.
Host side: shard/cast inputs, run SPMD on cores 0-7, concatenate outputs.
"""

import numpy as np
import ml_dtypes

bf16 = ml_dtypes.bfloat16

# Problem shape (hardcoded per contract)
B, T, S, C, D, V = 32, 64, 400, 100, 512, 32000
R = B * T              # 2048 rows, row r = t*32 + b
NC = 8
VS = V // NC           # 4000 vocab cols per core
PAD_IDX = 1
NEG_INF = -1e9

KCH = D // 128         # 4 contraction chunks of 128
NRB = R // 128         # 16 row blocks
NVT = 8                # vocab tiles of 500 per core
SCH = 4                # s-chunks of 100 for the copy einsum
CHUNKS = [(0, 4), (4, 8), (8, 12), (12, 16)]  # (start_rb, end_rb) per AR chunk
E_BUFS = 10            # e-tile slots
ST_BUFS = 6

_cache = {}


def _build(all_bias: bool):
    import concourse.bass as bass
    import concourse.mybir as mybir
    import concourse.tile as tile
    from concourse import bacc

    fp32 = mybir.dt.float32
    bf = mybir.dt.bfloat16
    AF = mybir.ActivationFunctionType

    nc = bacc.Bacc("TRN2", target_bir_lowering=False, debug=False, num_devices=NC)

    # ---- I/O ----
    hT_d = nc.dram_tensor("hTp", [128, KCH * R], bf, kind="ExternalInput")
    W_d = nc.dram_tensor("Wk", [NVT * 128, KCH * 500], bf, kind="ExternalInput")
    bias_d = nc.dram_tensor("biask", [1, VS], bf, kind="ExternalInput")
    wc_d = nc.dram_tensor("wc", [D, 1], bf, kind="ExternalInput")
    mask_d = nc.dram_tensor("maskm", [128, 2], fp32, kind="ExternalInput")
    attnT_d = nc.dram_tensor("attnT", [S, 256], bf, kind="ExternalInput")
    srcmap_d = nc.dram_tensor("srcmap", [S, 4 * C], bf, kind="ExternalInput")
    hTcp_d = nc.dram_tensor("hTcp", [D, 256], bf, kind="ExternalInput")
    out_d = nc.dram_tensor("out", [R, VS], bf, kind="ExternalOutput")
    cp_d = nc.dram_tensor("cp", [T, 4 * C], fp32, kind="ExternalOutput")

    rg = [list(range(NC))]

    with tile.TileContext(nc) as tc:
        with (
            tc.tile_pool(name="sb", bufs=1) as sb,
            tc.tile_pool(name="ps", bufs=4, space="PSUM") as ps,
            tc.tile_pool(name="dr", bufs=2, space="DRAM") as dr,
        ):
            # Dependency-free dummy AllReduce issued first: absorbs the
            # collective stack's ~50us cold-start while inputs stream in.
            warm_in = dr.tile([1, 256], fp32, bufs=1)
            warm_out = dr.tile([1, 256], fp32, bufs=1, addr_space="Shared")
            nc.gpsimd.collective_compute(
                "AllReduce", mybir.AluOpType.add,
                replica_groups=rg, ins=[warm_in.opt()], outs=[warm_out.opt()])

            # ---- resident loads ----
            # One fast sync queue in strict need-order: hT first (gates the
            # prologue and every rb0 matmul), then wc, then the W tiles in
            # first-use order.  Splitting across queues only splits the same
            # ~290GB/s, so priority order beats parallelism here.
            hT_sb = sb.tile([128, KCH, R], bf)
            nc.sync.dma_start(hT_sb[:, :, :], hT_d.ap().rearrange("p (k r) -> p k r", k=KCH))
            wc_sb = sb.tile([128, KCH], bf)
            nc.sync.dma_start(wc_sb[:, :], wc_d.ap().rearrange("(c p) one -> p (c one)", p=128))
            W_t = [sb.tile([128, KCH * 500], bf, name=f"W{vt}") for vt in range(NVT)]
            for vt in range(NVT):
                nc.sync.dma_start(W_t[vt][:, :], W_d.ap()[vt * 128:(vt + 1) * 128, :])
            # gpsimd: small tensors (done by ~10us, long before any trigger)
            mask_sb = sb.tile([128, 2], fp32)        # [:,0]=keep-mask, [:,1]=1-mask
            nc.gpsimd.dma_start(mask_sb[:, :], mask_d.ap())
            attnT_sb = sb.tile([100, SCH, 256], bf)
            nc.gpsimd.dma_start(attnT_sb[:, :, :], attnT_d.ap().rearrange("(c p) j -> p c j", p=100))
            srcmap_sb = sb.tile([100, SCH, 4 * C], bf)
            nc.gpsimd.dma_start(srcmap_sb[:, :, :], srcmap_d.ap().rearrange("(c p) j -> p c j", p=100))
            hTcp_sb = sb.tile([128, KCH, 256], bf)
            nc.gpsimd.dma_start(hTcp_sb[:, :, :], hTcp_d.ap().rearrange("(c p) j -> p c j", p=128))
            bias_sb = sb.tile([1, VS], bf)
            if all_bias:
                nc.gpsimd.dma_start(bias_sb[:, :], bias_d.ap())

            ones_sb = sb.tile([1, 128], bf)
            nc.vector.memset(ones_sb[:, :], 1.0)

            # ---- persistent small tiles ----
            pcT_sb = sb.tile([1, R], fp32)          # p_copy, row-major along free dim
            pc_sb = sb.tile([128, NRB], fp32)       # p_copy, [row%128, rowblock]
            rs_parts = sb.tile([128, 4, NRB], fp32) # rowsum quarters
            e1s_sb = sb.tile([128, NRB], fp32)      # masked exp(PAD) per row
            rs_sb = sb.tile([128, NRB], fp32)       # local rowsum
            z_sb = sb.tile([128, NRB], fp32)        # global rowsum
            zinv_sb = sb.tile([128, NRB], fp32)
            scale_sb = sb.tile([128, NRB], fp32)    # (1-p_copy)/Z
            pcTcp_sb = sb.tile([1, 256], bf)        # p_copy for this core's copy rows
            prep_sb = sb.tile([128, 256], fp32)     # p_copy bcast for copy path
            atts_sb = sb.tile([100, SCH, 256], bf)
            cp_sb = sb.tile([64, 4 * C], fp32)

            def emit_prologue():
                # p_copy for all rows (PE + sigmoid), placed after chunk 0
                for g in range(R // 512):
                    pps = ps.tile([1, 512], fp32, tag="stripe", name=f"pcT_ps{g}")
                    for kk in range(KCH):
                        nc.tensor.matmul(
                            pps[:, :], wc_sb[:, kk:kk + 1],
                            hT_sb[:, kk, g * 512:(g + 1) * 512],
                            start=(kk == 0), stop=(kk == KCH - 1))
                    nc.scalar.activation(pcT_sb[:, g * 512:(g + 1) * 512], pps[:, :], AF.Sigmoid)
                # copy-path head: p_copy for this core's rows + broadcast
                cps1 = ps.tile([1, 256], fp32, tag="stripe", name="cps1")
                for kk in range(KCH):
                    nc.tensor.matmul(
                        cps1[:, :], wc_sb[:, kk:kk + 1], hTcp_sb[:, kk, :],
                        start=(kk == 0), stop=(kk == KCH - 1))
                nc.scalar.activation(pcTcp_sb[:, :], cps1[:, :], AF.Sigmoid)
                prep = ps.tile([128, 256], fp32, tag="stripe", name="prep")
                nc.tensor.matmul(prep[:, :], ones_sb[:, :], pcTcp_sb[:, :],
                                 start=True, stop=True)
                nc.scalar.activation(prep_sb[:, :], prep[:, :], AF.Copy)

            def emit_cp_gpsimd():
                # attnT_scaled muls + p_copy partition scatter (gpsimd, after
                # chunk 0's trigger so they never delay it)
                for c in range(SCH):
                    nc.gpsimd.tensor_mul(atts_sb[:, c, :], attnT_sb[:, c, :],
                                         prep_sb[0:100, :])
                pcd = dr.tile([1, R], fp32, bufs=1)
                nc.gpsimd.dma_start(pcd[:, :], pcT_sb[:, :])
                nc.gpsimd.dma_start(pc_sb[:, :], pcd.rearrange("one (rb p) -> (one p) rb", p=128))

            # ================= main loop =================
            # prologue + copy-path head fill the input-DMA ramp: they need
            # only hT/wc/hTcp, which land before the W tiles.
            emit_prologue()
            emit_cp_gpsimd()
            # second dummy collective: resyncs the cores after the ramp so
            # chunk 0's AllGather doesn't pay the accumulated launch skew
            warm2_in = dr.tile([1, 64], fp32, bufs=1)
            warm2_out = dr.tile([8, 64], fp32, bufs=1, addr_space="Shared")
            nc.gpsimd.collective_compute(
                "AllGather", mybir.AluOpType.bypass,
                replica_groups=rg, ins=[warm2_in.opt()], outs=[warm2_out.opt()])
            e_tiles = []
            for ch, (rb0, rb1) in enumerate(CHUNKS):
                for rb in range(rb0, rb1):
                    et = sb.tile([128, VS], bf, tag="e", bufs=E_BUFS, name=f"e{rb}")
                    e_tiles.append(et)
                    for h in range(4):
                        stripe = ps.tile([128, 2, 512], fp32, tag="stripe", name=f"l{rb}_{h}")
                        for j in range(2):
                            vt = h * 2 + j
                            for kk in range(KCH):
                                nc.tensor.matmul(
                                    stripe[:, j, 0:500],
                                    hT_sb[:, kk, rb * 128:(rb + 1) * 128],
                                    W_t[vt][:, kk * 500:(kk + 1) * 500],
                                    start=(kk == 0),
                                    stop=(kk == KCH - 1 and not all_bias))
                            if all_bias:
                                nc.tensor.matmul(
                                    stripe[:, j, 0:500],
                                    ones_sb[:, :],
                                    bias_sb[:, vt * 500:(vt + 1) * 500],
                                    start=False, stop=True)
                        ev = et[:, h * 1000:(h + 1) * 1000].rearrange("p (g v) -> p g v", g=2)
                        nc.scalar.activation(
                            ev, stripe[:, :, 0:500], AF.Exp,
                            accum_out=rs_parts[:, h, rb:rb + 1])
                        if h == 0 and not all_bias:
                            # PAD mask: save exp(PAD) where masked, zero it
                            nc.vector.tensor_scalar_mul(
                                e1s_sb[:, rb:rb + 1], et[:, PAD_IDX:PAD_IDX + 1],
                                mask_sb[:, 1:2])
                            nc.vector.tensor_scalar_mul(
                                et[:, PAD_IDX:PAD_IDX + 1], et[:, PAD_IDX:PAD_IDX + 1],
                                mask_sb[:, 0:1])

                if ch == len(CHUNKS) - 1:
                    # copy-path einsum runs inside the last AR's latency
                    cpps = ps.tile([64, 4 * C], fp32, tag="stripe", name="cpps")
                    for bb in range(4):
                        for c in range(SCH):
                            nc.tensor.matmul(
                                cpps[:, bb * C:(bb + 1) * C],
                                atts_sb[:, c, bb * 64:(bb + 1) * 64],
                                srcmap_sb[:, c, bb * C:(bb + 1) * C],
                                start=(c == 0), stop=(c == SCH - 1))
                    nc.scalar.activation(cp_sb[:, :], cpps[:, :], AF.Copy)
                    nc.sync.dma_start(cp_d.ap(), cp_sb[:, :])

                # ---- chunk epilogue ----
                sl = slice(rb0, rb1)
                nrbc = rb1 - rb0
                # rowsum add + AR feeder + trigger on gpsimd: never gated by z
                nc.gpsimd.tensor_add(rs_sb[:, sl], rs_parts[:, 0, sl], rs_parts[:, 1, sl])
                nc.gpsimd.tensor_add(rs_sb[:, sl], rs_sb[:, sl], rs_parts[:, 2, sl])
                nc.gpsimd.tensor_add(rs_sb[:, sl], rs_sb[:, sl], rs_parts[:, 3, sl])
                if not all_bias:
                    nc.gpsimd.tensor_sub(rs_sb[:, sl], rs_sb[:, sl], e1s_sb[:, sl])
                ar_in = dr.tile([128, nrbc], fp32, tag=f"arin{ch}", bufs=1,
                                name=f"arin{ch}")
                ag_out = dr.tile([8, 128 * nrbc], fp32, tag=f"arout{ch}", bufs=1,
                                 addr_space="Shared", name=f"arout{ch}")
                nc.gpsimd.dma_start(ar_in[:, :], rs_sb[:, sl])
                # AllGather (bypass) costs the CC ~half an AllReduce; the
                # 7-way sum is trivial on DVE.
                nc.gpsimd.collective_compute(
                    "AllGather", mybir.AluOpType.bypass,
                    replica_groups=rg, ins=[ar_in.opt()], outs=[ag_out.opt()])
                # gathered readback on the sync queue, before the writes it gates
                g8 = sb.tile([128, 8, nrbc], fp32, tag=f"g8{ch}", bufs=1,
                             name=f"g8{ch}")
                nc.sync.dma_start(g8[:, :, :], ag_out.rearrange("r (p j) -> p r j", p=128))
                nc.vector.tensor_add(z_sb[:, sl], g8[:, 0, :], g8[:, 1, :])
                for r in range(2, 8):
                    nc.vector.tensor_add(z_sb[:, sl], z_sb[:, sl], g8[:, r, :])
                nc.vector.reciprocal(zinv_sb[:, sl], z_sb[:, sl])
                # scale = (1 - p_copy) * (1/Z)
                nc.vector.tensor_scalar(
                    out=scale_sb[:, sl], in0=pc_sb[:, sl], scalar1=-1.0, scalar2=1.0,
                    op0=mybir.AluOpType.mult, op1=mybir.AluOpType.add)
                nc.vector.tensor_mul(scale_sb[:, sl], scale_sb[:, sl], zinv_sb[:, sl])

                # ---- pass C (DVE): out = e * scale, stream to DRAM ----
                for rb in range(rb0, rb1):
                    et = e_tiles[rb]
                    sc = scale_sb[:, rb:rb + 1]
                    for h in range(2):
                        st = sb.tile([128, 2000], bf, tag="st", bufs=ST_BUFS,
                                     name=f"st{rb}_{h}")
                        nc.vector.tensor_scalar_mul(
                            st[:, :], et[:, h * 2000:(h + 1) * 2000], sc)
                        nc.sync.dma_start(
                            out_d.ap()[rb * 128:(rb + 1) * 128, h * 2000:(h + 1) * 2000],
                            st[:, :])

    nc.compile()
    return nc


def _get_nc(all_bias: bool):
    key = ("nc", all_bias)
    if key not in _cache:
        _cache[key] = _build(all_bias)
    return _cache[key]


def kernel(hidden, attn, src_map, W, b, Wc, bc):
    from concourse.bass_utils import run_bass_kernel_spmd

    hidden = np.asarray(hidden, dtype=np.float32)
    attn = np.asarray(attn, dtype=np.float32)
    src_map = np.asarray(src_map, dtype=np.float32)
    W = np.asarray(W, dtype=np.float32)
    b = np.asarray(b, dtype=np.float32)
    Wc = np.asarray(Wc, dtype=np.float32)
    bc = np.asarray(bc, dtype=np.float32)

    all_bias = bool(np.any(b != 0.0))

    bc_val = float(bc.reshape(-1)[0]) if bc.size else 0.0
    if bc_val != 0.0:
        raise NotImplementedError("bc != 0 not supported (bc is zero in this problem)")

    # hT packed: [p, kk*R + r] = hidden[r, kk*128+p]
    hTp = np.ascontiguousarray(
        hidden.T.reshape(KCH, 128, R).transpose(1, 0, 2).reshape(128, KCH * R)
    ).astype(bf16)
    wc = Wc.astype(bf16)                                          # [512, 1]

    nc = _get_nc(all_bias)

    in_maps = []
    for k in range(NC):
        Wk = W[:, k * VS:(k + 1) * VS]
        # pre-pack per-vt tiles: [vt*128+p, kk*500+c] = W[kk*128+p, vt*500+c]
        Wk = np.ascontiguousarray(
            Wk.reshape(KCH, 128, NVT, 500).transpose(2, 1, 0, 3).reshape(NVT * 128, KCH * 500)
        ).astype(bf16)
        bias_k = b[k * VS:(k + 1) * VS].astype(np.float64)
        if k == 0:
            bias_k = bias_k.copy()
            bias_k[PAD_IDX] += NEG_INF
        bias_k = bias_k.astype(bf16)[None, :]                     # [1, 4000]

        # PAD mask: core 0 zeroes its col PAD_IDX; others keep it
        m = 1.0 if k != 0 else 0.0
        mask_k = np.empty((128, 2), dtype=np.float32)
        mask_k[:, 0] = m
        mask_k[:, 1] = 1.0 - m

        # copy-path shard: batches 4k..4k+3, packed col j = bb*64 + t
        rows = np.array([[t * 32 + 4 * k + bb for t in range(T)] for bb in range(4)])
        rows_flat = rows.reshape(-1)
        attnT_k = np.ascontiguousarray(attn[rows_flat, :].T).astype(bf16)   # [400, 256]
        srcmap_k = np.ascontiguousarray(
            src_map[:, 4 * k:4 * k + 4, :].reshape(S, 4 * C)).astype(bf16)  # [400, 400]
        hTcp_k = np.ascontiguousarray(hidden[rows_flat, :].T).astype(bf16)  # [512, 256]

        in_maps.append({
            "hTp": hTp, "Wk": Wk, "biask": bias_k, "wc": wc, "maskm": mask_k,
            "attnT": attnT_k, "srcmap": srcmap_k, "hTcp": hTcp_k,
        })

    global _last_in_maps
    _last_in_maps = in_maps
    res = run_bass_kernel_spmd(nc, in_maps, core_ids=list(range(NC))).results

    full = np.empty((R, V + C), dtype=np.float32)
    t_idx = np.arange(T) * 32
    for k in range(NC):
        full[:, k * VS:(k + 1) * VS] = res[k]["out"]
        cp = res[k]["cp"].reshape(T, 4, C)
        for bb in range(4):
            full[t_idx + 4 * k + bb, V:] = cp[:, bb, :]
    return full


# revision 10
# speedup vs baseline: 1.0426x; 1.0426x over previous
"""CopyGenerator kernel for 8 Trainium2 NeuronCores.

Strategy: tensor-parallel over the vocab dimension.
  - Each core computes logits = hidden @ W[:, k*4000:(k+1)*4000] (bf16 matmul,
    fp32 accumulate), exp via ACT with fused row-sum (accum_out).
  - Softmax denominator: partial row-sums AllReduced across the 8 cores in
    pipelined chunks [6,6,3,1]; a warmup AllReduce absorbs cold-start cost.
  - Queue discipline: inputs split across three DGE queues (sync: W tiles,
    scalar: hT, vector: small tensors) for a fast ramp; gpsimd carries ONLY
    the row-sum adds / AR feeders / triggers so the last AR fires the moment
    its rows finish; z readbacks ride the sync queue right before the writes
    they gate.
  - PAD masking without bias matmuls: column 1's exp is zeroed via a
    per-core host mask and subtracted from the row-sum.
  - out_prob shard = e * (1 - p_copy) / Z written in bf16 (host upcasts).
  - p_copy prologue + copy-path head run right after chunk 0 (PE stays hot);
    the copy-path einsum runs inside the last AR's latency window.
Host side: shard/cast inputs, run SPMD on cores 0-7, concatenate outputs.
"""

import numpy as np
import ml_dtypes

bf16 = ml_dtypes.bfloat16

# Problem shape (hardcoded per contract)
B, T, S, C, D, V = 32, 64, 400, 100, 512, 32000
R = B * T              # 2048 rows, row r = t*32 + b
NC = 8
VS = V // NC           # 4000 vocab cols per core
PAD_IDX = 1
NEG_INF = -1e9

KCH = D // 128         # 4 contraction chunks of 128
NRB = R // 128         # 16 row blocks
NVT = 8                # vocab tiles of 500 per core
SCH = 4                # s-chunks of 100 for the copy einsum
CHUNKS = [(0, 4), (4, 8), (8, 12), (12, 16)]  # (start_rb, end_rb) per AR chunk
E_BUFS = 10            # e-tile slots
ST_BUFS = 6

_cache = {}


def _build(all_bias: bool):
    import concourse.bass as bass
    import concourse.mybir as mybir
    import concourse.tile as tile
    from concourse import bacc

    fp32 = mybir.dt.float32
    bf = mybir.dt.bfloat16
    AF = mybir.ActivationFunctionType

    nc = bacc.Bacc("TRN2", target_bir_lowering=False, debug=False, num_devices=NC)

    # ---- I/O ----
    hT_d = nc.dram_tensor("hTp", [128, KCH * R], bf, kind="ExternalInput")
    W_d = nc.dram_tensor("Wk", [NVT * 128, KCH * 500], bf, kind="ExternalInput")
    bias_d = nc.dram_tensor("biask", [1, VS], bf, kind="ExternalInput")
    wc_d = nc.dram_tensor("wc", [D, 1], bf, kind="ExternalInput")
    mask_d = nc.dram_tensor("maskm", [128, 2], fp32, kind="ExternalInput")
    attnT_d = nc.dram_tensor("attnT", [S, 256], bf, kind="ExternalInput")
    srcmap_d = nc.dram_tensor("srcmap", [S, 4 * C], bf, kind="ExternalInput")
    hTcp_d = nc.dram_tensor("hTcp", [D, 256], bf, kind="ExternalInput")
    out_d = nc.dram_tensor("out", [R, VS], bf, kind="ExternalOutput")
    cp_d = nc.dram_tensor("cp", [T, 4 * C], fp32, kind="ExternalOutput")

    rg = [list(range(NC))]

    with tile.TileContext(nc) as tc:
        with (
            tc.tile_pool(name="sb", bufs=1) as sb,
            tc.tile_pool(name="ps", bufs=4, space="PSUM") as ps,
            tc.tile_pool(name="dr", bufs=2, space="DRAM") as dr,
        ):
            # Dependency-free dummy AllReduce issued first: absorbs the
            # collective stack's ~50us cold-start while inputs stream in.
            warm_in = dr.tile([1, 256], fp32, bufs=1)
            warm_out = dr.tile([1, 256], fp32, bufs=1, addr_space="Shared")
            nc.gpsimd.collective_compute(
                "AllReduce", mybir.AluOpType.add,
                replica_groups=rg, ins=[warm_in.opt()], outs=[warm_out.opt()])

            # ---- resident loads ----
            # One fast sync queue in strict need-order: hT first (gates the
            # prologue and every rb0 matmul), then wc, then the W tiles in
            # first-use order.  Splitting across queues only splits the same
            # ~290GB/s, so priority order beats parallelism here.
            hT_sb = sb.tile([128, KCH, R], bf)
            nc.sync.dma_start(hT_sb[:, :, :], hT_d.ap().rearrange("p (k r) -> p k r", k=KCH))
            wc_sb = sb.tile([128, KCH], bf)
            nc.sync.dma_start(wc_sb[:, :], wc_d.ap().rearrange("(c p) one -> p (c one)", p=128))
            W_t = [sb.tile([128, KCH * 500], bf, name=f"W{vt}") for vt in range(NVT)]
            for vt in range(NVT):
                nc.sync.dma_start(W_t[vt][:, :], W_d.ap()[vt * 128:(vt + 1) * 128, :])
            # gpsimd: small tensors (done by ~10us, long before any trigger)
            mask_sb = sb.tile([128, 2], fp32)        # [:,0]=keep-mask, [:,1]=1-mask
            nc.gpsimd.dma_start(mask_sb[:, :], mask_d.ap())
            attnT_sb = sb.tile([100, SCH, 256], bf)
            nc.gpsimd.dma_start(attnT_sb[:, :, :], attnT_d.ap().rearrange("(c p) j -> p c j", p=100))
            srcmap_sb = sb.tile([100, SCH, 4 * C], bf)
            nc.gpsimd.dma_start(srcmap_sb[:, :, :], srcmap_d.ap().rearrange("(c p) j -> p c j", p=100))
            hTcp_sb = sb.tile([128, KCH, 256], bf)
            nc.gpsimd.dma_start(hTcp_sb[:, :, :], hTcp_d.ap().rearrange("(c p) j -> p c j", p=128))
            bias_sb = sb.tile([1, VS], bf)
            if all_bias:
                nc.gpsimd.dma_start(bias_sb[:, :], bias_d.ap())

            ones_sb = sb.tile([1, 128], bf)
            nc.vector.memset(ones_sb[:, :], 1.0)

            # ---- persistent small tiles ----
            pcT_sb = sb.tile([1, R], fp32)          # p_copy, row-major along free dim
            pc_sb = sb.tile([128, NRB], fp32)       # p_copy, [row%128, rowblock]
            rs_parts = sb.tile([128, 4, NRB], fp32) # rowsum quarters
            e1s_sb = sb.tile([128, NRB], fp32)      # masked exp(PAD) per row
            rs_sb = sb.tile([128, NRB], fp32)       # local rowsum
            z_sb = sb.tile([128, NRB], fp32)        # global rowsum
            zinv_sb = sb.tile([128, NRB], fp32)
            scale_sb = sb.tile([128, NRB], fp32)    # (1-p_copy)/Z
            pcTcp_sb = sb.tile([1, 256], bf)        # p_copy for this core's copy rows
            prep_sb = sb.tile([128, 256], fp32)     # p_copy bcast for copy path
            atts_sb = sb.tile([100, SCH, 256], bf)
            cp_sb = sb.tile([64, 4 * C], fp32)

            def emit_prologue():
                # p_copy for all rows (PE + sigmoid), placed after chunk 0
                for g in range(R // 512):
                    pps = ps.tile([1, 512], fp32, tag="stripe", name=f"pcT_ps{g}")
                    for kk in range(KCH):
                        nc.tensor.matmul(
                            pps[:, :], wc_sb[:, kk:kk + 1],
                            hT_sb[:, kk, g * 512:(g + 1) * 512],
                            start=(kk == 0), stop=(kk == KCH - 1))
                    nc.scalar.activation(pcT_sb[:, g * 512:(g + 1) * 512], pps[:, :], AF.Sigmoid)
                # copy-path head: p_copy for this core's rows + broadcast
                cps1 = ps.tile([1, 256], fp32, tag="stripe", name="cps1")
                for kk in range(KCH):
                    nc.tensor.matmul(
                        cps1[:, :], wc_sb[:, kk:kk + 1], hTcp_sb[:, kk, :],
                        start=(kk == 0), stop=(kk == KCH - 1))
                nc.scalar.activation(pcTcp_sb[:, :], cps1[:, :], AF.Sigmoid)
                prep = ps.tile([128, 256], fp32, tag="stripe", name="prep")
                nc.tensor.matmul(prep[:, :], ones_sb[:, :], pcTcp_sb[:, :],
                                 start=True, stop=True)
                nc.scalar.activation(prep_sb[:, :], prep[:, :], AF.Copy)

            def emit_cp_gpsimd():
                # attnT_scaled muls + p_copy partition scatter (gpsimd, after
                # chunk 0's trigger so they never delay it)
                for c in range(SCH):
                    nc.gpsimd.tensor_mul(atts_sb[:, c, :], attnT_sb[:, c, :],
                                         prep_sb[0:100, :])
                pcd = dr.tile([1, R], fp32, bufs=1)
                nc.gpsimd.dma_start(pcd[:, :], pcT_sb[:, :])
                nc.gpsimd.dma_start(pc_sb[:, :], pcd.rearrange("one (rb p) -> (one p) rb", p=128))

            # ================= main loop =================
            # prologue + copy-path head fill the input-DMA ramp: they need
            # only hT/wc/hTcp, which land before the W tiles.
            emit_prologue()
            emit_cp_gpsimd()
            e_tiles = []
            for ch, (rb0, rb1) in enumerate(CHUNKS):
                for rb in range(rb0, rb1):
                    et = sb.tile([128, VS], bf, tag="e", bufs=E_BUFS, name=f"e{rb}")
                    e_tiles.append(et)
                    for h in range(4):
                        stripe = ps.tile([128, 2, 512], fp32, tag="stripe", name=f"l{rb}_{h}")
                        for j in range(2):
                            vt = h * 2 + j
                            for kk in range(KCH):
                                nc.tensor.matmul(
                                    stripe[:, j, 0:500],
                                    hT_sb[:, kk, rb * 128:(rb + 1) * 128],
                                    W_t[vt][:, kk * 500:(kk + 1) * 500],
                                    start=(kk == 0),
                                    stop=(kk == KCH - 1 and not all_bias))
                            if all_bias:
                                nc.tensor.matmul(
                                    stripe[:, j, 0:500],
                                    ones_sb[:, :],
                                    bias_sb[:, vt * 500:(vt + 1) * 500],
                                    start=False, stop=True)
                        ev = et[:, h * 1000:(h + 1) * 1000].rearrange("p (g v) -> p g v", g=2)
                        nc.scalar.activation(
                            ev, stripe[:, :, 0:500], AF.Exp,
                            accum_out=rs_parts[:, h, rb:rb + 1])
                        if h == 0 and not all_bias:
                            # PAD mask: save exp(PAD) where masked, zero it
                            nc.vector.tensor_scalar_mul(
                                e1s_sb[:, rb:rb + 1], et[:, PAD_IDX:PAD_IDX + 1],
                                mask_sb[:, 1:2])
                            nc.vector.tensor_scalar_mul(
                                et[:, PAD_IDX:PAD_IDX + 1], et[:, PAD_IDX:PAD_IDX + 1],
                                mask_sb[:, 0:1])

                if ch == len(CHUNKS) - 1:
                    # copy-path einsum runs inside the last AR's latency
                    cpps = ps.tile([64, 4 * C], fp32, tag="stripe", name="cpps")
                    for bb in range(4):
                        for c in range(SCH):
                            nc.tensor.matmul(
                                cpps[:, bb * C:(bb + 1) * C],
                                atts_sb[:, c, bb * 64:(bb + 1) * 64],
                                srcmap_sb[:, c, bb * C:(bb + 1) * C],
                                start=(c == 0), stop=(c == SCH - 1))
                    nc.scalar.activation(cp_sb[:, :], cpps[:, :], AF.Copy)
                    nc.sync.dma_start(cp_d.ap(), cp_sb[:, :])

                # ---- chunk epilogue ----
                sl = slice(rb0, rb1)
                nrbc = rb1 - rb0
                # rowsum add + AR feeder + trigger on gpsimd: never gated by z
                nc.gpsimd.tensor_add(rs_sb[:, sl], rs_parts[:, 0, sl], rs_parts[:, 1, sl])
                nc.gpsimd.tensor_add(rs_sb[:, sl], rs_sb[:, sl], rs_parts[:, 2, sl])
                nc.gpsimd.tensor_add(rs_sb[:, sl], rs_sb[:, sl], rs_parts[:, 3, sl])
                if not all_bias:
                    nc.gpsimd.tensor_sub(rs_sb[:, sl], rs_sb[:, sl], e1s_sb[:, sl])
                ar_in = dr.tile([128, nrbc], fp32, tag=f"arin{ch}", bufs=1,
                                name=f"arin{ch}")
                ag_out = dr.tile([8, 128 * nrbc], fp32, tag=f"arout{ch}", bufs=1,
                                 addr_space="Shared", name=f"arout{ch}")
                nc.gpsimd.dma_start(ar_in[:, :], rs_sb[:, sl])
                # AllGather (bypass) costs the CC ~half an AllReduce; the
                # 7-way sum is trivial on DVE.
                nc.gpsimd.collective_compute(
                    "AllGather", mybir.AluOpType.bypass,
                    replica_groups=rg, ins=[ar_in.opt()], outs=[ag_out.opt()])
                # gathered readback on the sync queue, before the writes it gates
                g8 = sb.tile([128, 8, nrbc], fp32, tag=f"g8{ch}", bufs=1,
                             name=f"g8{ch}")
                nc.sync.dma_start(g8[:, :, :], ag_out.rearrange("r (p j) -> p r j", p=128))
                nc.vector.tensor_add(z_sb[:, sl], g8[:, 0, :], g8[:, 1, :])
                for r in range(2, 8):
                    nc.vector.tensor_add(z_sb[:, sl], z_sb[:, sl], g8[:, r, :])
                nc.vector.reciprocal(zinv_sb[:, sl], z_sb[:, sl])
                # scale = (1 - p_copy) * (1/Z)
                nc.vector.tensor_scalar(
                    out=scale_sb[:, sl], in0=pc_sb[:, sl], scalar1=-1.0, scalar2=1.0,
                    op0=mybir.AluOpType.mult, op1=mybir.AluOpType.add)
                nc.vector.tensor_mul(scale_sb[:, sl], scale_sb[:, sl], zinv_sb[:, sl])

                # ---- pass C (DVE): out = e * scale, stream to DRAM ----
                for rb in range(rb0, rb1):
                    et = e_tiles[rb]
                    sc = scale_sb[:, rb:rb + 1]
                    for h in range(2):
                        st = sb.tile([128, 2000], bf, tag="st", bufs=ST_BUFS,
                                     name=f"st{rb}_{h}")
                        nc.vector.tensor_scalar_mul(
                            st[:, :], et[:, h * 2000:(h + 1) * 2000], sc)
                        nc.sync.dma_start(
                            out_d.ap()[rb * 128:(rb + 1) * 128, h * 2000:(h + 1) * 2000],
                            st[:, :])

    nc.compile()
    return nc


def _get_nc(all_bias: bool):
    key = ("nc", all_bias)
    if key not in _cache:
        _cache[key] = _build(all_bias)
    return _cache[key]


def kernel(hidden, attn, src_map, W, b, Wc, bc):
    from concourse.bass_utils import run_bass_kernel_spmd

    hidden = np.asarray(hidden, dtype=np.float32)
    attn = np.asarray(attn, dtype=np.float32)
    src_map = np.asarray(src_map, dtype=np.float32)
    W = np.asarray(W, dtype=np.float32)
    b = np.asarray(b, dtype=np.float32)
    Wc = np.asarray(Wc, dtype=np.float32)
    bc = np.asarray(bc, dtype=np.float32)

    all_bias = bool(np.any(b != 0.0))

    bc_val = float(bc.reshape(-1)[0]) if bc.size else 0.0
    if bc_val != 0.0:
        raise NotImplementedError("bc != 0 not supported (bc is zero in this problem)")

    # hT packed: [p, kk*R + r] = hidden[r, kk*128+p]
    hTp = np.ascontiguousarray(
        hidden.T.reshape(KCH, 128, R).transpose(1, 0, 2).reshape(128, KCH * R)
    ).astype(bf16)
    wc = Wc.astype(bf16)                                          # [512, 1]

    nc = _get_nc(all_bias)

    in_maps = []
    for k in range(NC):
        Wk = W[:, k * VS:(k + 1) * VS]
        # pre-pack per-vt tiles: [vt*128+p, kk*500+c] = W[kk*128+p, vt*500+c]
        Wk = np.ascontiguousarray(
            Wk.reshape(KCH, 128, NVT, 500).transpose(2, 1, 0, 3).reshape(NVT * 128, KCH * 500)
        ).astype(bf16)
        bias_k = b[k * VS:(k + 1) * VS].astype(np.float64)
        if k == 0:
            bias_k = bias_k.copy()
            bias_k[PAD_IDX] += NEG_INF
        bias_k = bias_k.astype(bf16)[None, :]                     # [1, 4000]

        # PAD mask: core 0 zeroes its col PAD_IDX; others keep it
        m = 1.0 if k != 0 else 0.0
        mask_k = np.empty((128, 2), dtype=np.float32)
        mask_k[:, 0] = m
        mask_k[:, 1] = 1.0 - m

        # copy-path shard: batches 4k..4k+3, packed col j = bb*64 + t
        rows = np.array([[t * 32 + 4 * k + bb for t in range(T)] for bb in range(4)])
        rows_flat = rows.reshape(-1)
        attnT_k = np.ascontiguousarray(attn[rows_flat, :].T).astype(bf16)   # [400, 256]
        srcmap_k = np.ascontiguousarray(
            src_map[:, 4 * k:4 * k + 4, :].reshape(S, 4 * C)).astype(bf16)  # [400, 400]
        hTcp_k = np.ascontiguousarray(hidden[rows_flat, :].T).astype(bf16)  # [512, 256]

        in_maps.append({
            "hTp": hTp, "Wk": Wk, "biask": bias_k, "wc": wc, "maskm": mask_k,
            "attnT": attnT_k, "srcmap": srcmap_k, "hTcp": hTcp_k,
        })

    global _last_in_maps
    _last_in_maps = in_maps
    res = run_bass_kernel_spmd(nc, in_maps, core_ids=list(range(NC))).results

    full = np.empty((R, V + C), dtype=np.float32)
    t_idx = np.arange(T) * 32
    for k in range(NC):
        full[:, k * VS:(k + 1) * VS] = res[k]["out"]
        cp = res[k]["cp"].reshape(T, 4, C)
        for bb in range(4):
            full[t_idx + 4 * k + bb, V:] = cp[:, bb, :]
    return full
